# revision 5
# baseline (speedup 1.0000x reference)
"""Trainium2 Bass kernel for nn_DSQGAttentionN (banded sparse attention).

Sharding: 8 cores = 2 batches x 4 head-groups (4 heads each), all-fp16
matmul pipeline with fp32 PSUM accumulation.

Per-core device program (identical program across cores, data differs):
  A: qkT/kT [dh, tok] via matmul with host-permuted Wqkv columns
  B: V natural [tok, dv] with appended ones column (fused softmax denom)
  C: gateT = sigmoid(...)
  D: banded attention: per (head, 128-query block) only relative key
     chunks {0,1,2,3,4,6,8,12} contain any of the 44 taps. Transposed
     score tiles S^T[j,i]; tap/pos_bias mask via identity-matmul
     accumulate; exp on ScalarE; AV+denominator fused per chunk.
  E: normalize (approx reciprocal + ones-matmul broadcast), gate
     multiply, output projection -> partial y [2048, 1024] f32.
Host: sums the 4 head-group partials per batch, adds bout.
"""

import numpy as np

import concourse.bass as bass
import concourse.mybir as mybir
import concourse.tile as tile
from concourse import bacc
from concourse.bass_utils import run_bass_kernel_spmd
from concourse.masks import make_identity
from concourse.dve_ops import RECIP_APPROX_FAST_CONSTS, RECIPROCAL_APPROX_FAST

F32 = mybir.dt.float32
F16 = mybir.dt.float16

B, N, D, H = 2, 2048, 1024, 16
HD = D // H
HG = 4            # heads per core
NB = N // 128     # 16 query blocks
G = [0, 1, 2, 3, 4, 6, 8, 12]   # relative key chunks that contain taps
OFFSETS = sorted(set(range(0, 33)) | {48, 64, 96, 128, 192, 256, 384, 512, 768, 1024, 1536})
MASK_NEG = -30000.0
EXP_SHIFT = -3.0   # subtracted from scores (cancels in softmax); keeps exp small in fp16


def build_nc():
    nc = bacc.Bacc("TRN2", target_bir_lowering=False, debug=False)

    xT = nc.dram_tensor("xT", [128, 8, N], F16, kind="ExternalInput")
    wqk = nc.dram_tensor("wqk", [128, 8, 512], F16, kind="ExternalInput")
    wv = nc.dram_tensor("wv", [128, 8, 256], F16, kind="ExternalInput")
    wg = nc.dram_tensor("wg", [128, 8, 256], F16, kind="ExternalInput")
    wo = nc.dram_tensor("wo", [128, 2, D], F16, kind="ExternalInput")
    maskt = nc.dram_tensor("maskt", [128, HG, len(G), 128], F16, kind="ExternalInput")
    bqk2 = nc.dram_tensor("bqk2", [128, 4], F32, kind="ExternalInput")
    bg2 = nc.dram_tensor("bg2", [128, 2], F32, kind="ExternalInput")
    bv2 = nc.dram_tensor("bv2", [128, 2], F32, kind="ExternalInput")
    y = nc.dram_tensor("y", [N, D], F32, kind="ExternalOutput")

    with tile.TileContext(nc) as tc:
        with tc.tile_pool(name="persist", bufs=1) as persist:
            qkT = persist.tile([128, 4, N], F16)         # [part, (q01,q23,k01,k23), tok]
            vsb = persist.tile([128, NB, HG * 65], F16)  # V chunks; 65th col = ones
            gateT = persist.tile([128, 2, N], F16)
            wo_sb = persist.tile([128, 2, D], F16)
            maskt_sb = persist.tile([128, HG, len(G), 128], F16)
            bqk2_sb = persist.tile([128, 4], F32)
            bg2_sb = persist.tile([128, 2], F32)
            bv2_sb = persist.tile([128, 2], F32)
            ident = persist.tile([128, 128], F16)
            onesb = persist.tile([128, 64], F16)
            flatstage = persist.tile([64, HG, N], F16)
            denstage = persist.tile([65, HG, N], F32)
            fgstage = persist.tile([128, 2, N], F16)
            denr = persist.tile([65, 2 * N], F32)   # head h -> (row [0,32,64,0][h], col chunk h//3)
            recipr = persist.tile([65, 2 * N], F16)
            fgfinal = persist.tile([128, 2, N], F16)

            nc.sync.dma_start(out=wo_sb, in_=wo.ap())
            nc.sync.dma_start(out=maskt_sb, in_=maskt.ap())
            nc.sync.dma_start(out=bqk2_sb, in_=bqk2.ap())
            nc.sync.dma_start(out=bg2_sb, in_=bg2.ap())
            nc.sync.dma_start(out=bv2_sb, in_=bv2.ap())
            make_identity(nc, ident)
            nc.vector.memset(onesb, 1.0)
            nc.vector.memset(denr, 1.0)
            for h in range(HG):
                nc.vector.memset(vsb[:, :, 65 * h + 64:65 * h + 65], 1.0)

            with (
                tc.tile_pool(name="psproj", bufs=2, space="PSUM") as psproj,
                tc.tile_pool(name="psst", bufs=3, space="PSUM") as psst,
                tc.tile_pool(name="psav", bufs=2, space="PSUM") as psav,
                tc.tile_pool(name="dpool", bufs=3) as dpool,
                tc.tile_pool(name="ypool", bufs=3) as ypool,
            ):
                # ---- stages A-C: projections ----
                with tc.tile_pool(name="load", bufs=1) as load:
                    xT_sb = load.tile([128, 8, N], F16)
                    wqk_sb = load.tile([128, 8, 512], F16)
                    wv_sb = load.tile([128, 8, 256], F16)
                    wg_sb = load.tile([128, 8, 256], F16)
                    nc.sync.dma_start(out=xT_sb, in_=xT.ap())
                    nc.sync.dma_start(out=wqk_sb, in_=wqk.ap())
                    nc.sync.dma_start(out=wv_sb, in_=wv.ap())
                    nc.sync.dma_start(out=wg_sb, in_=wg.ap())

                    # A: qkT (q scaled by 1/8 on evac)
                    for gi in range(4):
                        for nt in range(4):
                            ps = psproj.tile([128, 512], F32, tag="proj")
                            for kc in range(8):
                                nc.tensor.matmul(
                                    ps,
                                    lhsT=wqk_sb[:, kc, gi * 128:(gi + 1) * 128],
                                    rhs=xT_sb[:, kc, nt * 512:(nt + 1) * 512],
                                    start=(kc == 0), stop=(kc == 7),
                                )
                            nc.scalar.activation(
                                qkT[:, gi, nt * 512:(nt + 1) * 512], ps,
                                mybir.ActivationFunctionType.Identity,
                                bias=bqk2_sb[:, gi:gi + 1],
                                scale=(HD ** -0.5) if gi < 2 else 1.0,
                            )

                    # B: V natural layout
                    for tci in range(NB):
                        psv = psproj.tile([128, 512], F32, tag="proj")
                        for kc in range(8):
                            nc.tensor.matmul(
                                psv[:, 0:256],
                                lhsT=xT_sb[:, kc, tci * 128:(tci + 1) * 128],
                                rhs=wv_sb[:, kc, :],
                                start=(kc == 0), stop=(kc == 7),
                            )
                        nc.scalar.activation(
                            vsb[:, tci, :].rearrange("p (h u) -> p h u", u=65)[:, :, 0:64],
                            psv[:, 0:256].rearrange("p (h u) -> p h u", u=64),
                            mybir.ActivationFunctionType.Copy,
                        )

                    # C: gateT
                    for gi2 in range(2):
                        for nt in range(4):
                            psg = psproj.tile([128, 512], F32, tag="proj")
                            for kc in range(8):
                                nc.tensor.matmul(
                                    psg,
                                    lhsT=wg_sb[:, kc, gi2 * 128:(gi2 + 1) * 128],
                                    rhs=xT_sb[:, kc, nt * 512:(nt + 1) * 512],
                                    start=(kc == 0), stop=(kc == 7),
                                )
                            nc.scalar.activation(
                                gateT[:, gi2, nt * 512:(nt + 1) * 512], psg,
                                mybir.ActivationFunctionType.Sigmoid,
                                bias=bg2_sb[:, gi2:gi2 + 1],
                            )

                # ---- stage D: banded attention ----
                for h in range(HG):
                    pq = 64 * (h % 2)
                    pg = h // 2
                    for qb in range(NB):
                        gs = [g for g in G if qb - g >= 0]  # prefix of G
                        expst = dpool.tile([128, len(G), 128], F16, tag="expst")
                        av = psav.tile([65, 128], F32, tag="av")
                        for gi, g in enumerate(gs):
                            m = qb - g
                            st = psst.tile([128, 128], F32, tag="st")
                            nc.tensor.matmul(
                                st, lhsT=ident, rhs=maskt_sb[:, h, gi, :],
                                start=True, stop=False,
                            )
                            nc.tensor.matmul(
                                st,
                                lhsT=qkT[pq:pq + 64, 2 + pg, m * 128:(m + 1) * 128],
                                rhs=qkT[pq:pq + 64, pg, qb * 128:(qb + 1) * 128],
                                start=False, stop=True,
                            )
                            nc.scalar.activation(
                                expst[:, gi, :], st, mybir.ActivationFunctionType.Exp,
                            )
                        for gi, g in enumerate(gs):
                            m = qb - g
                            nc.tensor.matmul(
                                av,
                                lhsT=vsb[:, m, 65 * h:65 * h + 65],
                                rhs=expst[:, gi, :],
                                start=(gi == 0), stop=(gi == len(gs) - 1),
                            )
                        nc.scalar.copy(
                            flatstage[0:64, h, qb * 128:(qb + 1) * 128], av[0:64, :])
                        nc.scalar.copy(
                            denstage[64:65, h, qb * 128:(qb + 1) * 128], av[64:65, :])
                    # repack this head's rows into gate-aligned layout
                    nc.sync.dma_start(
                        out=fgstage[pq:pq + 64, pg, :], in_=flatstage[0:64, h, :])
                    dr, dc = (32 * h, 0) if h < 3 else (0, 1)
                    nc.sync.dma_start(
                        out=denr[dr:dr + 1, dc * N:(dc + 1) * N], in_=denstage[64:65, h, :])

                # ---- stage E ----
                c = RECIP_APPROX_FAST_CONSTS
                nc.vector._custom_dve(
                    RECIPROCAL_APPROX_FAST, out=recipr, in0=denr,
                    s0=c["s0"], s1=c["s1"], imm2=c["imm2"],
                )
                for h in range(HG):
                    pq = 64 * (h % 2)
                    pg = h // 2
                    for nt in range(4):
                        rb = psproj.tile([128, 512], F32, tag="proj")
                        dr, dc = (32 * h, 0) if h < 3 else (0, 1)
                        nc.tensor.matmul(
                            rb[pq:pq + 64, :],
                            lhsT=onesb[dr:dr + 1, :],
                            rhs=recipr[dr:dr + 1, dc * N + nt * 512:dc * N + (nt + 1) * 512],
                            start=True, stop=True,
                        )
                        tmp = ypool.tile([128, 512], F16, tag="tmp")
                        nc.vector.tensor_mul(
                            tmp[pq:pq + 64, :],
                            fgstage[pq:pq + 64, pg, nt * 512:(nt + 1) * 512],
                            rb[pq:pq + 64, :],
                        )
                        nc.vector.scalar_tensor_tensor(
                            out=fgfinal[pq:pq + 64, pg, nt * 512:(nt + 1) * 512],
                            in0=tmp[pq:pq + 64, :],
                            scalar=bv2_sb[pq:pq + 64, pg:pg + 1],
                            in1=gateT[pq:pq + 64, pg, nt * 512:(nt + 1) * 512],
                            op0=mybir.AluOpType.add,
                            op1=mybir.AluOpType.mult,
                        )
                for tci in range(NB):
                    for nt2 in range(2):
                        psy = psproj.tile([128, 512], F32, tag="proj")
                        for kc2 in range(2):
                            nc.tensor.matmul(
                                psy,
                                lhsT=fgfinal[:, kc2, tci * 128:(tci + 1) * 128],
                                rhs=wo_sb[:, kc2, nt2 * 512:(nt2 + 1) * 512],
                                start=(kc2 == 0), stop=(kc2 == 1),
                            )
                        ysb = ypool.tile([128, 512], F32, tag="y")
                        nc.scalar.copy(ysb, psy)
                        nc.sync.dma_start(
                            out=y.ap()[tci * 128:(tci + 1) * 128,
                                       nt2 * 512:(nt2 + 1) * 512],
                            in_=ysb)

    nc.compile()
    return nc


def make_core_inputs(inputs, b, hg):
    x = np.asarray(inputs["x"], np.float32)
    Wqkv = np.asarray(inputs["Wqkv"], np.float32)
    bqkv = np.asarray(inputs["bqkv"], np.float32)
    Wgate = np.asarray(inputs["Wgate"], np.float32)
    bgate = np.asarray(inputs["bgate"], np.float32)
    Wout = np.asarray(inputs["Wout"], np.float32)
    pos_bias = np.asarray(inputs["pos_bias"], np.float32)

    H0 = HG * hg
    xT = np.ascontiguousarray(x[b].T).reshape(8, 128, N).transpose(1, 0, 2)

    cols = []
    for base in (0, D):   # q then k
        for hp in range(2):
            for hh in range(2):
                hglob = H0 + 2 * hp + hh
                cols.append(np.arange(base + 64 * hglob, base + 64 * hglob + 64))
    cols = np.concatenate(cols)
    wqk = Wqkv[:, cols].reshape(8, 128, 512).transpose(1, 0, 2)
    bqk2 = np.ascontiguousarray(bqkv[cols].reshape(4, 128).T)

    vcols = np.arange(2 * D + 64 * H0, 2 * D + 64 * H0 + 256)
    wv = Wqkv[:, vcols].reshape(8, 128, 256).transpose(1, 0, 2)
    bv2 = np.ascontiguousarray(bqkv[vcols].reshape(2, 128).T)

    gcols = np.arange(256 * hg, 256 * hg + 256)
    wg = Wgate[:, gcols].reshape(8, 128, 256).transpose(1, 0, 2)
    bg2 = np.ascontiguousarray(bgate[gcols].reshape(2, 128).T)

    wo = Wout[256 * hg:256 * hg + 256, :].reshape(2, 128, D).transpose(1, 0, 2)

    off_idx = {d: i for i, d in enumerate(OFFSETS)}
    jj = np.arange(128)[:, None]
    ii = np.arange(128)[None, :]
    maskt = np.full((128, HG, len(G), 128), MASK_NEG, np.float32)
    for gi, g in enumerate(G):
        delta = 128 * g + ii - jj
        base_m = np.full((128, 128), MASK_NEG, np.float32)
        sels = [(delta == dlt, oi) for dlt, oi in off_idx.items() if
                -127 <= dlt - 128 * g <= 127]
        for hl in range(HG):
            m = base_m.copy()
            for sel, oi in sels:
                m[sel] = pos_bias[oi, H0 + hl] + EXP_SHIFT
            maskt[:, hl, gi, :] = m

    f16c = lambda a: np.ascontiguousarray(a, np.float16)
    return dict(
        xT=f16c(xT), wqk=f16c(wqk), wv=f16c(wv), wg=f16c(wg), wo=f16c(wo),
        maskt=f16c(maskt),
        bqk2=bqk2.astype(np.float32), bg2=bg2.astype(np.float32),
        bv2=bv2.astype(np.float32),
    )


_CACHE = {}


def _get_nc():
    if "nc" not in _CACHE:
        _CACHE["nc"] = build_nc()
    return _CACHE["nc"]


def kernel(**inputs):
    nc = _get_nc()
    in_maps = [make_core_inputs(inputs, c // 4, c % 4) for c in range(8)]
    res = run_bass_kernel_spmd(nc, in_maps, core_ids=list(range(8)))
    bout = np.asarray(inputs["bout"], np.float32)
    out = np.zeros((B, N, D), np.float32)
    for c in range(8):
        out[c // 4] += res.results[c]["y"]
    out += bout
    return out
